# revision 41
# baseline (speedup 1.0000x reference)
"""Trainium2 Bass kernel for nn_DistanceProbeAlternative (retrieval_knn).

Computes, per batch b:
    proj = emb[b] @ W.T                      # [S, R]
    dist[i, j] = ||proj_i||^2 - 2 proj_i . proj_j + ||proj_j||^2

Sharding: data-parallel over batch B=32 across 8 cores (4 batches/core).
W is replicated. No collectives.

Host prep (inside kernel(), before the device launch): emb is cast to
fp16 (same rounding the device cast-DMA applied in earlier versions)
and laid out d-major (embT16 [b, d, s]) so the PE consumes it directly;
W is cast + blocked to WT16. The device writes the distance matrix in
fp16 and the host upcasts to fp32 (quantization ~3e-4 rel vs 2e-2 tol).

Per-core device dataflow (v4):
  1. embT s-halves (8 x 1MB) DMA'd on the gpsimd SWDGE queue (engine is
     otherwise idle; sync/scalar HWDGE triggers proved expensive), all
     issued up front; W on sync. 4 x 2MB embT resident in SBUF.
  2. projT[r, s] = sum_k WT_k.T @ embT_k (fp16 -> fp32 PSUM);
     projT fp16 (DVE copy); sq = projT*projT on DVE (f32r).
  3. norms: ncol[128,2/i] (sq x ones, fp32r N=2 rules) = +ni;
     nrow [1,S] = +nj; rank-1 rowrep [128,S] fp16 = +nj. Copies on DVE.
  4. dots i-tile: 2 matmuls into one [128,1024] 2-bank PSUM tile; ONE
     merged ACT tmp(fp16) = -2*psum + ncol; DVE add (all fp16)
     outsb = tmp + rowrep; fp16 out-DMA [128,1024] on sync.
  PE order: dots(b) pairs 0-1, proj(b+1), dots pairs 2-3, norms(b+1) --
  epilogue engines drain the pair 0-1 backlog during proj(b+1).
"""

import numpy as np
from contextlib import ExitStack

import concourse.bass as bass
import concourse.bacc as bacc
import concourse.tile as tile
from concourse import mybir
from concourse.bass_utils import run_bass_kernel_spmd

B, S, D, R = 32, 1024, 1024, 128
NCORES = 8
BPC = B // NCORES  # batches per core

F32 = mybir.dt.float32
F32R = mybir.dt.float32r
F16 = mybir.dt.float16
IDENT = mybir.ActivationFunctionType.Identity


def build_nc():
    nc = bacc.Bacc("TRN2", target_bir_lowering=False, debug=False)

    embTd = nc.dram_tensor("embT16", [BPC, D, S], F16, kind="ExternalInput")
    WTd = nc.dram_tensor("WT16", [128, D], F16, kind="ExternalInput")
    out = nc.dram_tensor("out16", [BPC, S, S], F16, kind="ExternalOutput")

    NST = S // 128  # 8 s-tiles per batch
    NDT = D // 128  # 8 d-tiles

    with tile.TileContext(nc) as tc, ExitStack() as ctx:
        constp = ctx.enter_context(tc.tile_pool(name="const", bufs=1))
        embT_p = ctx.enter_context(tc.tile_pool(name="embT", bufs=4))
        projT_p = ctx.enter_context(tc.tile_pool(name="projT", bufs=2))
        projTm2_p = ctx.enter_context(tc.tile_pool(name="projTm2", bufs=2))
        sq_p = ctx.enter_context(tc.tile_pool(name="sq", bufs=2))
        ncol_p = ctx.enter_context(tc.tile_pool(name="ncol", bufs=2))
        rowrep_p = ctx.enter_context(tc.tile_pool(name="rowrep", bufs=2))
        out_p = ctx.enter_context(tc.tile_pool(name="outsb", bufs=10))
        tmp_p = ctx.enter_context(tc.tile_pool(name="tmpsb", bufs=2))
        normps_p = ctx.enter_context(tc.tile_pool(name="normps", bufs=1, space="PSUM"))
        projps_p = ctx.enter_context(tc.tile_pool(name="projps", bufs=2, space="PSUM"))
        dotps_p = ctx.enter_context(tc.tile_pool(name="dotps", bufs=5, space="PSUM"))

        WT16 = constp.tile([128, D], F16, name="WT16")
        nc.sync.dma_start(out=WT16, in_=WTd.ap())

        # embT input: 2 s-half DMAs per batch on the (otherwise idle)
        # gpsimd SWDGE queue, all issued up front in consumption order.
        embTs = []
        for b in range(BPC):
            embT = embT_p.tile([128, NDT * S], F16, name="embT")
            embTs.append(embT)
            dst = embT.rearrange("p (k s) -> p k s", k=NDT)
            src = embTd.ap()[b, :, :].rearrange("(k p) s -> p k s", p=128)
            if b == 0:
                # batch 0 in s-quarters so proj(b0) pipelines with input
                for q in range(4):
                    nc.gpsimd.dma_start(
                        out=dst[:, :, 256 * q : 256 * (q + 1)],
                        in_=src[:, :, 256 * q : 256 * (q + 1)],
                    )
            else:
                for h in range(2):
                    nc.gpsimd.dma_start(
                        out=dst[:, :, 512 * h : 512 * (h + 1)],
                        in_=src[:, :, 512 * h : 512 * (h + 1)],
                    )

        onesf = constp.tile([128, 128], F32, name="onesf")
        nc.vector.memset(onesf, 1.0)
        ones = constp.tile([128, 128], F32R, name="ones")
        nc.vector.tensor_copy(ones, onesf)

        def proj_alloc():
            projT = projT_p.tile([128, S], F16, name="projT")
            projTm2 = projTm2_p.tile([128, S], F16, name="projTm2")
            sq = sq_p.tile([128, S], F32R, name="sq")
            return projT, projTm2, sq

        def proj_phase(embT, tiles, phases, nh=2):
            """Accumulating matmuls -> projT, projTm2 (=-2*projT) fp16 on
            Scalar + sq f32r on DVE. Emitting one phase at a time spreads
            PE filler work across the batch. nh=4 (256-wide) pipelines
            batch 0's proj with its incoming s-quarter DMAs."""
            w = S // nh
            projT, projTm2, sq = tiles
            for h in phases:
                projps = projps_p.tile([128, w], F32, name="projps")
                for k in range(NDT):
                    nc.tensor.matmul(
                        projps,
                        WT16[:, 128 * k : 128 * (k + 1)],
                        embT[:, S * k + w * h : S * k + w * (h + 1)],
                        start=(k == 0),
                        stop=(k == NDT - 1),
                    )
                sl = slice(w * h, w * (h + 1))
                nc.scalar.copy(projT[:, sl], projps)
                nc.scalar.mul(projTm2[:, sl], projps, -2.0)
                # sq from projT on DVE: projps frees after 2 Scalar ACTs
                nc.vector.tensor_mul(sq[:, sl], projT[:, sl], projT[:, sl])

        def norms_phase(sq):
            """ncol [128, 2/i-tile] f32 (+ni), rowrep [128,S] fp16 (+nj)."""
            # N=2 (ones cols) keeps the fp32r even-count/8B-alignment rules
            ncol_ps = normps_p.tile([128, 512], F32, tag="np", name="ncol_ps")
            for i in range(NST):
                nc.tensor.matmul(
                    ncol_ps[:, 2 * i : 2 * i + 2],
                    sq[:, 128 * i : 128 * (i + 1)],
                    ones[:, 0:2],
                    start=True,
                    stop=True,
                )
            ncol = ncol_p.tile([128, 2 * NST], F32, name="ncol")
            nc.vector.tensor_copy(ncol, ncol_ps[:, 0 : 2 * NST])

            # rowrep[m, j] = sum_r sq[r, j] for every partition m, in ONE
            # matmul per half: J.T @ sq with J = all-ones [128, 128]
            rowrep = rowrep_p.tile([128, S], F16, name="rowrep")
            for h in range(2):
                rp_ps = normps_p.tile([128, 512], F32, tag="np", name="rp_ps")
                nc.tensor.matmul(
                    rp_ps,
                    ones,
                    sq[:, 512 * h : 512 * (h + 1)],
                    start=True,
                    stop=True,
                )
                nc.scalar.copy(rowrep[:, 512 * h : 512 * (h + 1)], rp_ps)
            return ncol, rowrep

        ADD = mybir.AluOpType.add

        def dots_tile(b, i, projT, projTm2, ncol, rowrep, hybrid=False):
            """Upper-triangle i-tile: blocks j >= i only (host mirrors).

            d_ps = -2 * dot (projTm2 stationary). Default epilogue: ONE
            fused scalar_tensor_tensor per chunk on DVE:
            out = (rowrep + ncol) + d_ps. With hybrid=True (last batch's
            big tiles, where there is no proj/norms work to overlap) the
            tile instead uses Scalar ACT + DVE fp16 add, halving the DVE
            drain at the kernel tail.
            """
            j0 = 128 * i
            W = S - j0
            outsb = out_p.tile([128, 1024], F16, name="outsb")
            tmp = tmp_p.tile([128, 1024], F16, name="tmp") if hybrid else None
            off = 0
            while off < W:
                w = min(512, W - off)
                d_ps = dotps_p.tile([128, w], F32, tag="dp", name="d_ps")
                nc.tensor.matmul(
                    d_ps,
                    projTm2[:, 128 * i : 128 * (i + 1)],
                    projT[:, j0 + off : j0 + off + w],
                    start=True,
                    stop=True,
                )
                if hybrid:
                    # d_ps already holds -2*dot: scale=+1, bias=+ni
                    nc.scalar.activation(
                        tmp[:, off : off + w], d_ps, IDENT,
                        bias=ncol[:, 2 * i : 2 * i + 1], scale=1.0,
                    )
                else:
                    nc.vector.scalar_tensor_tensor(
                        out=outsb[:, off : off + w],
                        in0=rowrep[:, j0 + off : j0 + off + w],
                        scalar=ncol[:, 2 * i : 2 * i + 1],
                        in1=d_ps,
                        op0=ADD,
                        op1=ADD,
                    )
                off += w
            if hybrid:
                nc.vector.tensor_add(
                    outsb[:, 0:W], tmp[:, 0:W], rowrep[:, j0:S]
                )
            dram_dst = out.ap()[b, j0 : j0 + 128, j0:S]
            # two HWDGE rings so the output drain is not serialized behind
            # one FIFO; scalar (busier engine) takes only 2 triggers/batch
            eng = nc.scalar if i in (0, 2) else nc.sync
            eng.dma_start(out=dram_dst, in_=outsb[:, 0:W])

        tiles = proj_alloc()
        proj_phase(embTs[0], tiles, range(4), nh=4)
        projT, projTm2, sq = tiles
        ncol, rowrep = norms_phase(sq)

        # big/small tiles alternate: evens the DVE stt load and PSUM-bank
        # pressure across the batch instead of a small-tile burst at the end
        TILE_ORDER = [0, 4, 1, 5, 2, 6, 3, 7]

        for b in range(BPC):
            last = b + 1 >= BPC
            for idx, i in enumerate(TILE_ORDER):
                dots_tile(b, i, projT, projTm2, ncol, rowrep)
                if not last:
                    if idx == 1:
                        tiles_n = proj_alloc()
                        proj_phase(embTs[b + 1], tiles_n, [0])
                    elif idx == 3:
                        proj_phase(embTs[b + 1], tiles_n, [1])
                    elif idx == 5:
                        ncol_n, rowrep_n = norms_phase(tiles_n[2])
            if not last:
                projT, projTm2, sq = tiles_n
                ncol, rowrep = ncol_n, rowrep_n

    nc.finalize()
    return nc


_NC_CACHE = None


def _get_nc():
    global _NC_CACHE
    if _NC_CACHE is None:
        _NC_CACHE = build_nc()
    return _NC_CACHE


def _host_wt16(W):
    # WT16[p, 128k + j] = W[j, 128k + p]  (W^T in [d-part, k, r] blocks)
    Wf = np.asarray(W, dtype=np.float32)
    wt = Wf.T.reshape(8, 128, 128).transpose(1, 0, 2).reshape(128, 1024)
    return np.ascontiguousarray(wt).astype(np.float16)


def run(embeddings_batch, W, trace=False, tmpdir=None):
    nc = _get_nc()
    emb16 = np.asarray(embeddings_batch, dtype=np.float32).astype(np.float16)
    wt16 = _host_wt16(W)
    in_maps = [
        {
            "embT16": np.ascontiguousarray(
                emb16[c * BPC : (c + 1) * BPC].transpose(0, 2, 1)
            ),
            "WT16": wt16,
        }
        for c in range(NCORES)
    ]
    res = run_bass_kernel_spmd(
        nc, in_maps, core_ids=list(range(NCORES)), trace=trace, tmpdir=tmpdir
    )
    full = np.concatenate([r["out16"] for r in res.results], axis=0)
    # device wrote only blocks j >= i; mirror the strict lower blocks
    NB = S // 128
    M = full.reshape(B, NB, 128, NB, 128)
    iu = np.triu_indices(NB, 1)
    M[:, iu[1], :, iu[0], :] = M[:, iu[0], :, iu[1], :].swapaxes(-1, -2)
    return full.astype(np.float32), res


def kernel(embeddings_batch, W):
    full, _ = run(embeddings_batch, W, trace=False)
    return full
